# revision 1
# baseline (speedup 1.0000x reference)
"""Trainium2 Bass kernel for nn_Attention2d (sparse_attention) — v3.

Math (per reference):
  x: (2, 128, 64, 64); T = 4096 tokens; 4 heads x 32 channels.
  qkv 1x1-conv -> per-head attention over T -> 1x1-conv out proj -> residual.

Sharding: one (batch, head) pair per core (8 cores). Each core computes its
head's attention fully on-chip and returns the per-head partial of the
output projection (bf16); the host sums the 4 head partials per batch and
adds the residual + biases (exact, fp32).

Structure (v3):
  - Scores are produced in supers of 3 s-chunks (1536 fp32 cols = 3 PSUM
    banks), with the st tile DOUBLE-buffered (6 banks): the PE runs QK for
    super n+1 while ScalarE/VectorE exp super n — no serial QK<->exp chain.
  - exp is SPLIT: ScalarE does activation-Exp on cols [0:ES); VectorE does
    a Schraudolph exp2 bit-trick (one tensor_scalar mult+add writing int16
    bit patterns that ARE the bf16 of 2^(y/128)) on the rest. Both carry
    the same global 2^((C2-16256)/128) factor, which cancels in softmax.
  - PV: one matmul per chunk into a [64,512] fp32 accumulator (32 v rows +
    32 ones rows giving the denominator), 32 accumulations per t-block.
  - epilogue per t-block: ScalarE copies the accumulator out of PSUM,
    GpSimd copies the denominator rows, VectorE fast-reciprocal +
    normalize-mul, PE out-projection, ScalarE output copy, bf16 DMA out
    (host accumulates partials in fp32).
  - all inputs arrive in 3 packed DMAs (x / weights / biases).
"""

import numpy as np
import ml_dtypes

B, C, Hh, Ww = 2, 128, 64, 64
T = Hh * Ww          # 4096
NH, CH = 4, 32
SCALE2 = float(1.0 / np.sqrt(CH))
N_CORES = 8
NCH = 32             # s-chunks (of 128 tokens) per t-block

# Schraudolph exp2 constants (VectorE path): bits = round(raw*C1 + C2),
# bits viewed as bf16 == 2^((bits-16256)/128) ~= exp(raw*SCALE2) * 2^((C2-16256)/128)
C1 = SCALE2 * float(np.log2(np.e)) * 128.0
C2 = 16256.0 - 7.4
# ScalarE path matches the global factor so both halves share one scale:
SBIAS = float((C2 - 16256.0) / 128.0 * np.log(2.0))

# super layout within a t-block: 10x 3-chunk + 1x 2-chunk
SUPERS = [(c, min(3, NCH - c)) for c in range(0, NCH, 3)]
ES3, ES2 = 864, 576  # ScalarE exp cols for 1536/1024-col supers

_cache = {}


def _build_nc(debug=False, zero_bias=False):
    import concourse.tile as tile
    from concourse import bacc, mybir

    BF16 = mybir.dt.bfloat16
    F32 = mybir.dt.float32
    I16 = mybir.dt.int16
    Exp = mybir.ActivationFunctionType.Exp
    MULT = mybir.AluOpType.mult
    ADD = mybir.AluOpType.add

    nc = bacc.Bacc("TRN2", target_bir_lowering=False, debug=False,
                   num_devices=N_CORES)
    dbg = {}
    if debug:
        dbg["q"] = nc.dram_tensor("dq", [128, T], BF16, kind="ExternalOutput")
        dbg["k"] = nc.dram_tensor("dk", [128, T], BF16, kind="ExternalOutput")
        dbg["vT"] = nc.dram_tensor("dvT", [128, 2048], BF16,
                                   kind="ExternalOutput")
        dbg["pv"] = nc.dram_tensor("dpv", [64, 512], F32,
                                   kind="ExternalOutput")
        dbg["rc"] = nc.dram_tensor("drc", [32, 512], F32,
                                   kind="ExternalOutput")
        dbg["an"] = nc.dram_tensor("dan", [32, T], BF16,
                                   kind="ExternalOutput")

    x_in = nc.dram_tensor("x", [128, T], BF16, kind="ExternalInput")
    # packed weights: wqT | wkT | wvT | wpT(padded to 128 rows)
    w_in = nc.dram_tensor("wpack", [128, 416], BF16, kind="ExternalInput")
    b_in = nc.dram_tensor("bpack", [128, 2], F32, kind="ExternalInput")
    out_t = nc.dram_tensor("out", [128, T], BF16, kind="ExternalOutput")

    with tile.TileContext(nc) as tc:
        with (
            tc.tile_pool(name="const", bufs=1) as cpool,
            tc.tile_pool(name="work", bufs=2) as wpool,
            tc.tile_pool(name="psum", bufs=1, space="PSUM") as pspool,
        ):
            x_sb = cpool.tile([128, T], BF16)
            nc.sync.dma_start(x_sb[:], x_in[:])
            w_sb = cpool.tile([128, 416], BF16)
            nc.sync.dma_start(w_sb[:], w_in[:])
            b_sb = cpool.tile([128, 2], F32)
            nc.sync.dma_start(b_sb[:], b_in[:])
            wq_sb = w_sb[:, 0:128]
            wk_sb = w_sb[:, 128:256]
            wv_sb = w_sb[:, 256:288]
            wp_sb = w_sb[0:32, 288:416]
            bq_sb = b_sb[:, 0:1]
            bk_sb = b_sb[:, 1:2]

            q_sb = cpool.tile([128, T], BF16)
            k_sb = cpool.tile([128, T], BF16)
            vT_sb = cpool.tile([128, 64 * NCH], BF16)  # (128, 2048)

            nc.gpsimd.memset(vT_sb[:], 1.0)
            sbias_sb = cpool.tile([128, 1], F32)
            nc.gpsimd.memset(sbias_sb[:], SBIAS)

            # 1-bank fp32 scratch for projections / vT / out-proj
            def scratch(nm):
                return pspool.tile([128, 512], F32, tag="pp", bufs=1, name=nm)

            def emit_proj(wsb, bsb, dst, c, nm, eng=None):
                ps = scratch(nm)
                nc.tensor.matmul(ps[:], wsb,
                                 x_sb[:, c * 512:(c + 1) * 512],
                                 start=True, stop=True)
                if zero_bias and eng == "s":
                    nc.scalar.copy(dst[:, c * 512:(c + 1) * 512], ps[:])
                else:
                    nc.vector.tensor_scalar_add(
                        dst[:, c * 512:(c + 1) * 512], ps[:], bsb)

            def emit_vt(half):
                ps = scratch(f"pp_v{half}")
                for j16 in range(16):
                    j = half * 16 + j16
                    nc.tensor.matmul(
                        ps[:, j16 * 32:(j16 + 1) * 32],
                        x_sb[:, j * 128:(j + 1) * 128],
                        wv_sb,
                        start=True, stop=True)
                src = ps[:].rearrange("p (j c) -> p j c", c=32)
                dstv = vT_sb[:].rearrange("p (j c) -> p j c", c=64)
                nc.vector.tensor_copy(
                    dstv[:, half * 16:(half + 1) * 16, 0:32], src)

            # ---- prologue: only what super 0 needs ----
            emit_proj(wk_sb, bk_sb, k_sb, 0, "pp_k0")
            emit_proj(wq_sb, bq_sb, q_sb, 0, "pp_q0")

            # ---- attention, software-pipelined over supers ----
            state = {}
            pv_tiles = {}
            an_tiles = {}
            supers_all = [(tb, c0, nch) for tb in range(8)
                          for (c0, nch) in SUPERS]

            def emit_score_exp(idx):
                tb, c0, nch = supers_all[idx]
                ncols = nch * 512
                es = ES3 if nch == 3 else ES2
                st = pspool.tile([128, 1536], F32, tag="st", bufs=2,
                                 name=f"st_{idx}")
                tsl = slice(tb * 512, (tb + 1) * 512)
                for ci in range(nch):
                    ch = c0 + ci
                    sblk = tb * 0 + ch  # s-chunk index (global over s)
                    nc.tensor.matmul(
                        st[:, ci * 512:(ci + 1) * 512],
                        k_sb[32 * (ch % 4):32 * (ch % 4) + 32,
                             128 * ch:128 * (ch + 1)],
                        q_sb[32 * (ch % 4):32 * (ch % 4) + 32, tsl],
                        start=True, stop=True,
                        tile_position=(32 * (ch % 4), 0))
                p_sb = wpool.tile([128, 1536], BF16, tag="p", bufs=4)
                nc.scalar.activation(p_sb[:, 0:es], st[:, 0:es], Exp,
                                     bias=sbias_sb[:], scale=SCALE2)
                nc.vector.tensor_scalar(
                    p_sb[:, es:ncols].bitcast(I16), st[:, es:ncols],
                    C1, C2, MULT, ADD)
                state[idx] = p_sb

            def emit_pv(idx):
                tb, c0, nch = supers_all[idx]
                p_sb = state.pop(idx)
                if c0 == 0:
                    pv_tiles[tb] = pspool.tile(
                        [64, 512], F32, tag="pv", bufs=1, name=f"pv_{tb}")
                pv = pv_tiles[tb]
                for ci in range(nch):
                    ch = c0 + ci
                    nc.tensor.matmul(
                        pv[:],
                        vT_sb[:, 64 * ch:64 * (ch + 1)],
                        p_sb[:, ci * 512:(ci + 1) * 512],
                        start=(ch == 0), stop=(ch == NCH - 1),
                        skip_group_check=True)
                if c0 + nch == NCH:
                    # t-block epilogue part A: ONE ScalarE copy is pv's
                    # only reader (so the accumulator frees fast for the
                    # next t-block); GpSimd re-bases the denominator rows
                    # (off the hot FIFOs), VectorE reciprocal + normalize
                    # mul. The out-projection (part B) is emitted several
                    # supers later so it never blocks the in-order PE
                    # queue while this chain completes.
                    a_h = wpool.tile([64, 512], F32, tag="ah")
                    nc.scalar.copy(a_h[:], pv[:])
                    dcp = wpool.tile([32, 512], F32, tag="dcp")
                    nc.gpsimd.tensor_copy(dcp[:], a_h[32:64, :])
                    rc = wpool.tile([32, 512], F32, tag="rc")
                    nc.vector.reciprocal_approx_fast(rc[:], dcp[:])
                    an_t = wpool.tile([32, 512], BF16, tag="an")
                    nc.vector.tensor_mul(an_t[:], a_h[0:32, :], rc[:])
                    an_tiles[tb] = an_t
                    if debug and tb == 0:
                        dpv_sb = wpool.tile([64, 512], F32, tag="dbgpv")
                        nc.vector.tensor_copy(dpv_sb[:], pv[:])
                        nc.sync.dma_start(dbg["pv"][:], dpv_sb[:])
                        nc.sync.dma_start(dbg["rc"][:], rc[:])
                        nc.sync.dma_start(dbg["an"][:, 0:512], an_t[:])

            def emit_store(tb):
                an_t = an_tiles.pop(tb)
                op = scratch(f"pp_o{tb}")
                nc.tensor.matmul(op[:], wp_sb, an_t[:],
                                 start=True, stop=True)
                o_sb = wpool.tile([128, 512], BF16, tag="o")
                nc.scalar.copy(o_sb[:], op[:])
                nc.sync.dma_start(
                    out_t[:, tb * 512:(tb + 1) * 512], o_sb[:])

            for idx in range(len(supers_all)):
                emit_score_exp(idx)
                if idx >= 2:
                    emit_pv(idx - 2)
                # stagger input prep into the supers that have slack,
                # ahead of the super that needs it
                if idx == 0:
                    emit_vt(0)
                if idx == 1:
                    emit_vt(1)
                if 0 <= idx <= 6:
                    emit_proj(wk_sb, bk_sb, k_sb, idx + 1, f"pp_k{idx + 1}",
                              eng="s" if idx % 2 == 0 else None)
                if idx % 11 == 5 and idx < 77:
                    c = idx // 11 + 1
                    emit_proj(wq_sb, bq_sb, q_sb, c, f"pp_q{c}",
                              eng="s" if c % 2 == 0 else None)
                # deferred epilogue part B: several supers after part A,
                # so the an_t chain has completed by the time the PE
                # dequeues the out-projection
                if idx % 11 == 6 and idx >= 11:
                    emit_store(idx // 11 - 1)
            emit_pv(len(supers_all) - 2)
            emit_pv(len(supers_all) - 1)
            emit_store(7)
            if debug:
                nc.sync.dma_start(dbg["q"][:], q_sb[:])
                nc.sync.dma_start(dbg["k"][:], k_sb[:])
                nc.sync.dma_start(dbg["vT"][:], vT_sb[:])

    nc.compile()
    return nc


def _get_nc(debug=False, zero_bias=False):
    key = ("nc", debug, zero_bias)
    if key not in _cache:
        _cache[key] = _build_nc(debug, zero_bias)
    return _cache[key]


def _make_in_maps(x_, w_qkv, b_qkv, w_proj):
    bf16 = ml_dtypes.bfloat16
    in_maps = []
    for core in range(N_CORES):
        b, g = divmod(core, NH)
        wq = w_qkv[96 * g:96 * g + 32]
        wk = w_qkv[96 * g + 32:96 * g + 64]
        wv = w_qkv[96 * g + 64:96 * g + 96]
        wpack = np.zeros((128, 416), np.float32)
        wpack[:, 0:128] = np.tile(wq, (4, 1)).T
        wpack[:, 128:256] = np.tile(wk, (4, 1)).T
        wpack[:, 256:288] = wv.T
        wpack[0:32, 288:416] = w_proj[:, 32 * g:32 * (g + 1)].T
        bpack = np.stack([np.tile(b_qkv[96 * g:96 * g + 32], 4),
                          np.tile(b_qkv[96 * g + 32:96 * g + 64], 4)],
                         axis=1)
        in_maps.append({
            "x": x_[b].astype(bf16),
            "wpack": np.ascontiguousarray(wpack).astype(bf16),
            "bpack": np.ascontiguousarray(bpack.astype(np.float32)),
        })
    return in_maps


def _run(x, w_qkv, b_qkv, w_proj, b_proj, trace=False):
    from concourse.bass_utils import run_bass_kernel_spmd

    x_ = np.ascontiguousarray(np.asarray(x, np.float32).reshape(B, C, T))
    w_qkv = np.asarray(w_qkv, np.float32)
    b_qkv = np.asarray(b_qkv, np.float32)
    w_proj = np.asarray(w_proj, np.float32)
    b_proj = np.asarray(b_proj, np.float32)
    nc = _get_nc(zero_bias=not np.any(b_qkv))

    in_maps = _make_in_maps(x_, w_qkv, b_qkv, w_proj)
    res = run_bass_kernel_spmd(nc, in_maps, core_ids=list(range(N_CORES)),
                               trace=trace)
    out = np.empty((B, C, T), np.float32)
    for b in range(B):
        acc = x_[b] + b_proj[:, None]
        for g in range(NH):
            wp = w_proj[:, 32 * g:32 * (g + 1)]
            bv = b_qkv[96 * g + 64:96 * g + 96]
            acc = acc + res.results[NH * b + g]["out"].astype(np.float32) \
                + (wp @ bv)[:, None]
        out[b] = acc
    return out.reshape(B, C, Hh, Ww), res


def kernel(x, w_qkv, b_qkv, w_proj, b_proj):
    out, _ = _run(x, w_qkv, b_qkv, w_proj, b_proj, trace=False)
    return out.astype(np.asarray(x).dtype)

